# revision 18
# baseline (speedup 1.0000x reference)
"""Trainium2 Bass kernel for nn_GCNNDiagGaussianActor.

Structural insight: the reference GNN runs GCNConv layers over a COMPLETE
graph of 32 nodes per sample with self-loops. Every node has degree exactly
32 and the symmetric GCN norm is 1/32 for every edge, so the gather +
segment_sum collapses to a per-graph mean broadcast to every node. Per graph:

    pooled = sum_n obs[g, n, 0:16]                   (cols 0:2 zeroed in W1)
    h1  = relu(pooled @ (W1/32) + b1)
    h2  = relu(h1 @ W2 + b2)
    m   = relu(h2 @ Wm1 + bm1)
    o   = m @ Wm2r + bm2r                            -> [128] per graph
    mu  = o[0:64];  std = exp(3.5 * tanh(o[64:128]) - 1.5)

Sharding: data-parallel over batch, 128 graphs per core = 128 SBUF
partitions; small weights replicated. The x32 node replication is folded
into the last GEMM by tiling Wm2's columns host-side.

Perf structure (v3 fp32 baseline ~22.3us -> ~17.1us measured same-state;
the ~13us floor is fixed NEFF overhead: ~6.5us wrapper semaphore-clear
epilogue + ~1.1us start barrier + ~2.2us minimum latency per DMA chain,
measured empirically with a trivial copy kernel):
- whole datapath in bf16 (harness rel-err gate is 2e-2; this kernel lands
  ~1e-3): obs DMA halves to 128KB/core and every matmul runs 1 cycle/row
  instead of fp32's 4 (no LOW_HIGH double pass).
- obs is split 72/56 across the SP and ACT HWDGE queue sets: the two
  ~630ns issue costs AND the transfers run in parallel (skew compensates
  ACT's ~134ns-slower DGE kick). mm1's weights ride second on SP (small,
  lands just before mm1); the fat W2|Wm1|Wm2r pack rides Pool's SWDGE,
  whose ~1us generation overlaps the window start and whose consumers
  (mm2+) run ~1.5us after mm1.
- node pooling: two contiguous bf16 adds (512->256->128) on DVE, then ONE
  PE transpose of the [128,128] partial-sum block into PSUM, a DVE bf16
  copy back to SBUF, and the remaining 8-way node sum folded into the
  first matmul by host-tiling W1 rows to K=128 (replaces the strided
  reduce + 4 stream transposes of earlier versions, ~1us faster).
- the transpose identity is built on-device (Pool iota, DVE is_equal)
  so it never competes with input DMA bandwidth.
- zero-bias fast path (this workload's biases are all zeros): relus are
  bare tensor_scalar_max on DVE. General path keeps b1/b2/bm1 as fp32
  bias columns fused into the relus and bm2 as a rank-1 PE accumulate.
- final GEMM split into log_std and mu halves on separate PSUM tiles
  (a shared tile creates a false reader-reader dep): tanh fires right
  after the ls half; the mu copy + SP DMA overlap tanh/exp; the std DMA
  issues from ACT the moment exp retires.
- PE p-state pre-warm: dummy matmuls on a junk tile keep the tensor
  engine busy through the obs DMA wait so real matmuls run at high clock.
- dummy tanh right after the DMA issues hoists the scalar engine's
  ACT_TABLE_LOAD (~1.3us) off the critical path.
- custom TileContext exit: drop the on-device semaphore RANGE_CLEAR and
  second exit barrier (the NEFF wrapper epilogue clears every semaphore
  after the kernel anyway), ~0.4us off the tail.
"""

import numpy as np

NCORES = 8
BS = 1024
BS_LOCAL = BS // NCORES   # 128 graphs per core
NN = 32                   # nodes per graph
FD = 16                   # per-node obs width
OBS_W = NN * FD           # 512
H = 128                   # hidden width
OUT_W = 2 * NN            # 64 = ACT_DIM * NN
# wpk cols: W2 | Wm1 | Wm2r | bm2ls row (w1til8 ships separately on SP;
# the transpose identity is built on-device)
C_W2 = 0
C_WM1 = H
C_WM2 = 2 * H
C_BM2 = 3 * H            # row 0 only, 64 wide
WPK = 3 * H + OUT_W
N_WARM_MM = 4             # PE p-state warm-up matmuls

_NC_CACHE = {}


def _build_bass(with_bias):
    import concourse.bacc as bacc
    import concourse.mybir as mybir
    from concourse import tile
    from concourse.vector_clock import ScopedClock

    class FastExitTileContext(tile.TileContext):
        # The stock exit emits drain + barrier + gpsimd sem RANGE_CLEAR +
        # second barrier. The NEFF wrapper epilogue clears every semaphore
        # after the kernel regardless, so the on-device clear and the
        # second barrier are pure tail latency here; keep only the
        # python-side allocator bookkeeping.
        def _drain_and_barrier(self, tick_clock, wait_clock):
            drain_inst = self.nc.sync.drain()
            wait_clock.add_sem_waits(
                drain_inst.ins, ScopedClock({None: tick_clock.global_clock})
            )
            self.nc.all_engine_barrier()
            popped = self.nc._tile_sem_poison_stack.pop()
            assert popped is self._sem_poison
            sems = list(self.sems.allocated().values())
            if sems:
                sem_nums = [
                    s.num if hasattr(s, "num") else s for s in sems
                ]
                self.nc._state.prepend_free_semaphores(sem_nums)
                for poison_set in self.nc._tile_sem_poison_stack:
                    poison_set.update(sem_nums)

    fp32 = mybir.dt.float32
    bf16 = mybir.dt.bfloat16
    AF = mybir.ActivationFunctionType
    ALU = mybir.AluOpType

    nc = bacc.Bacc(None, target_bir_lowering=False)
    obs = nc.declare_dram_parameter("obs", [BS_LOCAL, OBS_W], bf16, isOutput=False)
    wpk = nc.declare_dram_parameter("wpk", [H, WPK], bf16, isOutput=False)
    w1t = nc.declare_dram_parameter("w1t", [H, H], bf16, isOutput=False)
    if with_bias:
        bcols = nc.declare_dram_parameter("bcols", [H, 4], fp32, isOutput=False)
    out = nc.declare_dram_parameter("out", [2, BS_LOCAL, OUT_W], fp32, isOutput=True)

    with FastExitTileContext(nc) as tc:
        with (
            tc.tile_pool(name="sb", bufs=1) as pool,
            tc.tile_pool(name="ps", bufs=1, space="PSUM") as ppool,
        ):
            obs_t = pool.tile([BS_LOCAL, OBS_W], bf16)
            wp = pool.tile([H, WPK], bf16)
            w1_t = pool.tile([H, H], bf16)
            # obs split across SP and ACT HWDGE queue sets (parallel issue
            # AND parallel transfer); mm1's weights ride second on SP (small,
            # arrives just before mm1 needs them); the rest of the weights
            # ride Pool's SWDGE, whose ~1us generation overlaps the window
            # start and whose consumers (mm2+) run later
            HB = 72  # skewed split: ACT's DGE kick is ~134ns slower than SP's
            nc.sync.dma_start(obs_t[0:HB, :], obs[0:HB, :])
            nc.scalar.dma_start(obs_t[HB:BS_LOCAL, :], obs[HB:BS_LOCAL, :])
            nc.sync.dma_start(w1_t[:], w1t[:])
            nc.gpsimd.dma_start(wp[:], wpk[:])
            if with_bias:
                bc = pool.tile([H, 4], fp32)
                nc.sync.dma_start(bc[:], bcols[:])

            # transpose identity built on-device: Pool iota (col - row),
            # DVE is_equal -> 1.0 on the diagonal; both engines idle here
            iot = pool.tile([H, H], mybir.dt.int32)
            nc.gpsimd.iota(iot[:], pattern=[[1, H]], channel_multiplier=-1)
            ident = pool.tile([H, H], bf16)
            nc.vector.tensor_scalar(ident[:], iot[:], 0, None, ALU.is_equal)

            # junk tile for PE warm-up + small constants (DVE, off-path)
            junk = pool.tile([BS_LOCAL, OBS_W], bf16)
            nc.vector.memset(junk[:], 0.0)
            if with_bias:
                ones = pool.tile([1, H], bf16)
                nc.vector.memset(ones[:], 1.0)
            cm15 = pool.tile([BS_LOCAL, 1], fp32)
            nc.vector.memset(cm15[:], -1.5)
            warm = pool.tile([1, 1], fp32)
            nc.vector.memset(warm[:], 0.0)
            # dummy transcendental: hoists ACT_TABLE_LOAD into the DMA wait
            nc.scalar.activation(warm[:], warm[:], AF.Tanh)

            # PE p-state warm-up during the obs DMA wait; the last short
            # matmul keeps PE busy up to the A2 transpose arrival
            jp = ppool.tile([1, OBS_W], fp32)
            for _ in range(N_WARM_MM):
                nc.tensor.matmul(jp[:], junk[:, 0:1], junk[:], start=True, stop=True)
            nc.tensor.matmul(jp[:, 0:128], junk[:, 0:1], junk[:, 0:128], start=True, stop=True)

            # Node pooling, stage 1: 512 -> 256 -> 128 contiguous bf16 adds.
            # A2[g, n1*16+d] = sum over nodes n = n1 (mod 8) of obs dim d.
            A1 = pool.tile([BS_LOCAL, 256], bf16)
            nc.vector.tensor_add(A1[:], obs_t[:, 0:256], obs_t[:, 256:512])
            A2 = pool.tile([BS_LOCAL, 128], bf16)
            nc.vector.tensor_add(A2[:], A1[:, 0:128], A1[:, 128:256])

            # Stage 2: one PE transpose (vs 4 DVE stream transposes), then
            # the 8-way node sum rides inside mm1 via host-tiled W1 (K=128).
            tp = ppool.tile([H, BS_LOCAL], bf16)
            nc.tensor.transpose(tp[:], A2[:], ident[:])
            B = pool.tile([H, BS_LOCAL], bf16)
            nc.vector.tensor_copy(B[:], tp[:])

            # MLP chain: [ch, graphs] tiles, weights as lhsT, relu+bias
            # fused on DVE (out = max(psum + b, 0)).
            def relu(dst, src, col):
                if with_bias:
                    nc.vector.tensor_scalar(
                        dst, src, bc[:, col : col + 1], 0.0, ALU.add, ALU.max
                    )
                else:
                    nc.vector.tensor_scalar_max(dst, src, 0.0)

            h1_ps = ppool.tile([H, BS_LOCAL], fp32)
            nc.tensor.matmul(h1_ps[:], w1_t[:], B[:], start=True, stop=True)
            h1 = pool.tile([H, BS_LOCAL], bf16)
            relu(h1[:], h1_ps[:], 0)

            h2_ps = ppool.tile([H, BS_LOCAL], fp32)
            nc.tensor.matmul(h2_ps[:], wp[:, C_W2 : C_W2 + H], h1[:], start=True, stop=True)
            h2 = pool.tile([H, BS_LOCAL], bf16)
            relu(h2[:], h2_ps[:], 1)

            m_ps = ppool.tile([H, BS_LOCAL], fp32)
            nc.tensor.matmul(m_ps[:], wp[:, C_WM1 : C_WM1 + H], h2[:], start=True, stop=True)
            m = pool.tile([H, BS_LOCAL], bf16)
            relu(m[:], m_ps[:], 2)

            # Final layer, graphs on PSUM partitions. log_std half first
            # (+ rank-1 bm2 accumulate) so tanh starts earlier; mu half
            # second, its copy/DMA overlapping tanh/exp.
            # separate PSUM tiles for the two halves: a shared tile would
            # put a false reader-reader dep between tanh and the mu copy
            ls_ps = ppool.tile([BS_LOCAL, OUT_W], fp32)
            mu_ps = ppool.tile([BS_LOCAL, OUT_W], fp32)
            nc.tensor.matmul(
                ls_ps[:],
                m[:],
                wp[:, C_WM2 + OUT_W : C_WM2 + 2 * OUT_W],
                start=True,
                stop=not with_bias,
            )
            if with_bias:
                nc.tensor.matmul(
                    ls_ps[:],
                    ones[:],
                    wp[0:1, C_BM2 : C_BM2 + OUT_W],
                    start=False,
                    stop=True,
                )
            nc.tensor.matmul(
                mu_ps[:],
                m[:],
                wp[:, C_WM2 : C_WM2 + OUT_W],
                start=True,
                stop=True,
            )

            # std = exp(3.5*tanh(ls) - 1.5); DMA from ACT right after exp
            O = pool.tile([BS_LOCAL, 2 * OUT_W], fp32)
            tls = pool.tile([BS_LOCAL, OUT_W], fp32)
            nc.scalar.activation(tls[:], ls_ps[:], AF.Tanh)
            nc.vector.tensor_copy(O[:, 0:OUT_W], mu_ps[:])
            nc.sync.dma_start(out[0], O[:, 0:OUT_W])
            nc.scalar.activation(
                O[:, OUT_W : 2 * OUT_W], tls[:], AF.Exp, bias=cm15[:], scale=3.5
            )
            nc.scalar.dma_start(out[1], O[:, OUT_W : 2 * OUT_W])

    nc.compile()
    return nc


def _get_nc(with_bias):
    key = ("bias" if with_bias else "fast")
    if key not in _NC_CACHE:
        _NC_CACHE[key] = _build_bass(with_bias)
    return _NC_CACHE[key]


def _prep_inputs(inputs):
    import ml_dtypes

    bf16 = ml_dtypes.bfloat16

    obs = np.asarray(inputs["obs"], dtype=np.float32)
    W1 = np.asarray(inputs["W1"], dtype=np.float32)
    b1 = np.asarray(inputs["b1"], dtype=np.float32)
    W2 = np.asarray(inputs["W2"], dtype=np.float32)
    b2 = np.asarray(inputs["b2"], dtype=np.float32)
    Wm1 = np.asarray(inputs["Wm1"], dtype=np.float32)
    bm1 = np.asarray(inputs["bm1"], dtype=np.float32)
    Wm2 = np.asarray(inputs["Wm2"], dtype=np.float32)
    bm2 = np.asarray(inputs["bm2"], dtype=np.float32)

    # GCN symmetric norm over the complete-graph-with-self-loops: 1/32 per
    # edge; folded into W1. Layer 2's mean over identical node features is
    # the identity, so W2 is used as-is. w1til8 tiles W1 rows 8x so mm1
    # (K=128) also performs the final 8-way node sum of the pooling tree.
    w1b = np.zeros((FD, H), np.float32)
    w1b[2:FD] = W1 * np.float32(1.0 / 32.0)     # drops robot_loc cols 0:2
    w1til8 = np.tile(w1b, (8, 1))               # [128, 128]

    Wm2r = np.concatenate([np.tile(Wm2[:, 0:2], NN), np.tile(Wm2[:, 2:4], NN)], axis=1)
    bm2pad = np.zeros((H, OUT_W), np.float32)
    bm2pad[0] = np.tile(bm2[2:4], NN)           # log_std-plane bias row

    wpk = np.ascontiguousarray(
        np.concatenate([W2, Wm1, Wm2r, bm2pad], axis=1).astype(bf16)
    )
    bcols = np.ascontiguousarray(
        np.stack([b1, b2, bm1, np.zeros(H, np.float32)], axis=1)
    )

    with_bias = bool(
        np.any(b1) or np.any(b2) or np.any(bm1) or np.any(bm2[2:4])
    )
    obs16 = np.ascontiguousarray(obs.astype(bf16))
    shared = {"wpk": wpk, "w1t": np.ascontiguousarray(w1til8.astype(bf16))}
    if with_bias:
        shared["bcols"] = bcols
    in_maps = []
    for c in range(NCORES):
        mm = dict(shared)
        mm["obs"] = obs16[c * BS_LOCAL : (c + 1) * BS_LOCAL]
        in_maps.append(mm)
    return in_maps, with_bias


def kernel(**inputs):
    from concourse.bass_utils import run_bass_kernel_spmd

    assert inputs["obs"].shape == (BS, OBS_W), inputs["obs"].shape
    in_maps, with_bias = _prep_inputs(inputs)
    nc = _get_nc(with_bias)
    res = run_bass_kernel_spmd(nc, in_maps, list(range(NCORES))).results
    out = np.empty((2, BS, OUT_W), np.float32)
    for c in range(NCORES):
        out[:, c * BS_LOCAL : (c + 1) * BS_LOCAL, :] = res[c]["out"]
    # mu-plane bm2 is outside every nonlinearity -> exact host add
    bm2 = np.asarray(inputs["bm2"], dtype=np.float32)
    if bm2[0] != 0.0 or bm2[1] != 0.0:
        out[0] += np.tile(bm2[0:2], NN)[None, :]
    return out


# revision 19
# speedup vs baseline: 1.0082x; 1.0082x over previous
"""Trainium2 Bass kernel for nn_GCNNDiagGaussianActor.

Structural insight: the reference GNN runs GCNConv layers over a COMPLETE
graph of 32 nodes per sample with self-loops. Every node has degree exactly
32 and the symmetric GCN norm is 1/32 for every edge, so the gather +
segment_sum collapses to a per-graph mean broadcast to every node. Per graph:

    pooled = sum_n obs[g, n, 0:16]                   (cols 0:2 zeroed in W1)
    h1  = relu(pooled @ (W1/32) + b1)
    h2  = relu(h1 @ W2 + b2)
    m   = relu(h2 @ Wm1 + bm1)
    o   = m @ Wm2r + bm2r                            -> [128] per graph
    mu  = o[0:64];  std = exp(3.5 * tanh(o[64:128]) - 1.5)

Sharding: data-parallel over batch, 128 graphs per core = 128 SBUF
partitions; small weights replicated. The x32 node replication is folded
into the last GEMM by tiling Wm2's columns host-side.

Perf structure (v3 fp32 baseline ~22.3us -> ~17.1us measured same-state;
the ~13us floor is fixed NEFF overhead: ~6.5us wrapper semaphore-clear
epilogue + ~1.1us start barrier + ~2.2us minimum latency per DMA chain,
measured empirically with a trivial copy kernel):
- whole datapath in bf16 (harness rel-err gate is 2e-2; this kernel lands
  ~1e-3): obs DMA halves to 128KB/core and every matmul runs 1 cycle/row
  instead of fp32's 4 (no LOW_HIGH double pass).
- obs is split 72/56 across the SP and ACT HWDGE queue sets: the two
  ~630ns issue costs AND the transfers run in parallel (skew compensates
  ACT's ~134ns-slower DGE kick). mm1's weights ride second on SP (small,
  lands just before mm1); the fat W2|Wm1|Wm2r pack rides Pool's SWDGE,
  whose ~1us generation overlaps the window start and whose consumers
  (mm2+) run ~1.5us after mm1.
- node pooling: two contiguous bf16 adds (512->256->128) on DVE, then ONE
  PE transpose of the [128,128] partial-sum block into PSUM, a DVE bf16
  copy back to SBUF, and the remaining 8-way node sum folded into the
  first matmul by host-tiling W1 rows to K=128 (replaces the strided
  reduce + 4 stream transposes of earlier versions, ~1us faster).
- the transpose identity is built on-device (Pool iota, DVE is_equal)
  so it never competes with input DMA bandwidth.
- zero-bias fast path (this workload's biases are all zeros): relus are
  bare tensor_scalar_max on DVE. General path keeps b1/b2/bm1 as fp32
  bias columns fused into the relus and bm2 as a rank-1 PE accumulate.
- final GEMM split into log_std and mu halves on separate PSUM tiles
  (a shared tile creates a false reader-reader dep): tanh fires right
  after the ls half; the mu copy + SP DMA overlap tanh/exp; the std DMA
  issues from ACT the moment exp retires.
- PE p-state pre-warm: dummy matmuls on a junk tile keep the tensor
  engine busy through the obs DMA wait so real matmuls run at high clock.
- dummy tanh right after the DMA issues hoists the scalar engine's
  ACT_TABLE_LOAD (~1.3us) off the critical path.
- custom TileContext exit: drop the on-device semaphore RANGE_CLEAR and
  second exit barrier (the NEFF wrapper epilogue clears every semaphore
  after the kernel anyway), ~0.4us off the tail.
"""

import numpy as np

NCORES = 8
BS = 1024
BS_LOCAL = BS // NCORES   # 128 graphs per core
NN = 32                   # nodes per graph
FD = 16                   # per-node obs width
OBS_W = NN * FD           # 512
H = 128                   # hidden width
OUT_W = 2 * NN            # 64 = ACT_DIM * NN
# wpk cols: W2 | Wm1 | Wm2r | bm2ls row (w1til8 ships separately on SP;
# the transpose identity is built on-device)
C_W2 = 0
C_WM1 = H
C_WM2 = 2 * H
C_BM2 = 3 * H            # row 0 only, 64 wide
WPK = 3 * H + OUT_W
N_WARM_MM = 4             # PE p-state warm-up matmuls

_NC_CACHE = {}


def _build_bass(with_bias):
    import concourse.bacc as bacc
    import concourse.mybir as mybir
    from concourse import tile
    from concourse.vector_clock import ScopedClock

    class FastExitTileContext(tile.TileContext):
        # The stock exit emits drain + barrier + gpsimd sem RANGE_CLEAR +
        # second barrier. The NEFF wrapper epilogue clears every semaphore
        # after the kernel regardless, so the on-device clear and the
        # second barrier are pure tail latency here; keep only the
        # python-side allocator bookkeeping.
        def _drain_and_barrier(self, tick_clock, wait_clock):
            drain_inst = self.nc.sync.drain()
            wait_clock.add_sem_waits(
                drain_inst.ins, ScopedClock({None: tick_clock.global_clock})
            )
            self.nc.all_engine_barrier()
            popped = self.nc._tile_sem_poison_stack.pop()
            assert popped is self._sem_poison
            sems = list(self.sems.allocated().values())
            if sems:
                sem_nums = [
                    s.num if hasattr(s, "num") else s for s in sems
                ]
                self.nc._state.prepend_free_semaphores(sem_nums)
                for poison_set in self.nc._tile_sem_poison_stack:
                    poison_set.update(sem_nums)

    fp32 = mybir.dt.float32
    bf16 = mybir.dt.bfloat16
    AF = mybir.ActivationFunctionType
    ALU = mybir.AluOpType

    nc = bacc.Bacc(None, target_bir_lowering=False)
    obs = nc.declare_dram_parameter("obs", [BS_LOCAL, OBS_W], bf16, isOutput=False)
    wpk = nc.declare_dram_parameter("wpk", [H, WPK], bf16, isOutput=False)
    w1t = nc.declare_dram_parameter("w1t", [H, H], bf16, isOutput=False)
    if with_bias:
        bcols = nc.declare_dram_parameter("bcols", [H, 4], fp32, isOutput=False)
    out = nc.declare_dram_parameter("out", [2, BS_LOCAL, OUT_W], fp32, isOutput=True)

    with FastExitTileContext(nc) as tc:
        with (
            tc.tile_pool(name="sb", bufs=1) as pool,
            tc.tile_pool(name="ps", bufs=1, space="PSUM") as ppool,
        ):
            obs_t = pool.tile([BS_LOCAL, OBS_W], bf16)
            wp = pool.tile([H, WPK], bf16)
            w1_t = pool.tile([H, H], bf16)
            # obs split across SP and ACT HWDGE queue sets (parallel issue
            # AND parallel transfer); mm1's weights ride second on SP (small,
            # arrives just before mm1 needs them); the rest of the weights
            # ride Pool's SWDGE, whose ~1us generation overlaps the window
            # start and whose consumers (mm2+) run later
            HB = 72  # skewed split: ACT's DGE kick is ~134ns slower than SP's
            nc.sync.dma_start(obs_t[0:HB, :], obs[0:HB, :])
            nc.scalar.dma_start(obs_t[HB:BS_LOCAL, :], obs[HB:BS_LOCAL, :])
            nc.sync.dma_start(w1_t[:], w1t[:])
            nc.gpsimd.dma_start(wp[:], wpk[:])
            if with_bias:
                bc = pool.tile([H, 4], fp32)
                nc.sync.dma_start(bc[:], bcols[:])

            # transpose identity built on-device: Pool iota (col - row),
            # DVE is_equal -> 1.0 on the diagonal; both engines idle here
            iot = pool.tile([H, H], mybir.dt.int32)
            nc.gpsimd.iota(iot[:], pattern=[[1, H]], channel_multiplier=-1)
            ident = pool.tile([H, H], bf16)
            nc.vector.tensor_scalar(ident[:], iot[:], 0, None, ALU.is_equal)

            # junk tile for PE warm-up + small constants (DVE, off-path)
            junk = pool.tile([BS_LOCAL, OBS_W], bf16)
            nc.vector.memset(junk[:], 0.0)
            if with_bias:
                ones = pool.tile([1, H], bf16)
                nc.vector.memset(ones[:], 1.0)
            cm15 = pool.tile([BS_LOCAL, 1], fp32)
            nc.vector.memset(cm15[:], -1.5)
            warm = pool.tile([1, 1], fp32)
            nc.vector.memset(warm[:], 0.0)
            # dummy transcendental: hoists ACT_TABLE_LOAD into the DMA wait
            nc.scalar.activation(warm[:], warm[:], AF.Tanh)

            # PE p-state warm-up during the obs DMA wait; the last short
            # matmul keeps PE busy up to the A2 transpose arrival
            jp = ppool.tile([1, OBS_W], fp32)
            for _ in range(N_WARM_MM):
                nc.tensor.matmul(jp[:], junk[:, 0:1], junk[:], start=True, stop=True)
            nc.tensor.matmul(jp[:, 0:128], junk[:, 0:1], junk[:, 0:128], start=True, stop=True)

            # Node pooling, stage 1: 512 -> 256 -> 128 contiguous bf16 adds.
            # A2[g, n1*16+d] = sum over nodes n = n1 (mod 8) of obs dim d.
            A1 = pool.tile([BS_LOCAL, 256], bf16)
            nc.vector.tensor_add(A1[:], obs_t[:, 0:256], obs_t[:, 256:512])
            A2 = pool.tile([BS_LOCAL, 128], bf16)
            nc.vector.tensor_add(A2[:], A1[:, 0:128], A1[:, 128:256])

            # Stage 2: one PE transpose (vs 4 DVE stream transposes), then
            # the 8-way node sum rides inside mm1 via host-tiled W1 (K=128).
            tp = ppool.tile([H, BS_LOCAL], bf16)
            nc.tensor.transpose(tp[:], A2[:], ident[:])
            B = pool.tile([H, BS_LOCAL], bf16)
            nc.vector.tensor_copy(B[:], tp[:])

            # MLP chain: [ch, graphs] tiles, weights as lhsT, relu+bias
            # fused on DVE (out = max(psum + b, 0)).
            def relu(dst, src, col):
                if with_bias:
                    nc.vector.tensor_scalar(
                        dst, src, bc[:, col : col + 1], 0.0, ALU.add, ALU.max
                    )
                else:
                    nc.vector.tensor_scalar_max(dst, src, 0.0)

            h1_ps = ppool.tile([H, BS_LOCAL], fp32)
            nc.tensor.matmul(h1_ps[:], w1_t[:], B[:], start=True, stop=True)
            h1 = pool.tile([H, BS_LOCAL], bf16)
            relu(h1[:], h1_ps[:], 0)

            h2_ps = ppool.tile([H, BS_LOCAL], fp32)
            nc.tensor.matmul(h2_ps[:], wp[:, C_W2 : C_W2 + H], h1[:], start=True, stop=True)
            h2 = pool.tile([H, BS_LOCAL], bf16)
            relu(h2[:], h2_ps[:], 1)

            m_ps = ppool.tile([H, BS_LOCAL], fp32)
            nc.tensor.matmul(m_ps[:], wp[:, C_WM1 : C_WM1 + H], h2[:], start=True, stop=True)
            m = pool.tile([H, BS_LOCAL], bf16)
            relu(m[:], m_ps[:], 2)

            # Final layer, graphs on PSUM partitions. log_std half first
            # (+ rank-1 bm2 accumulate) so tanh starts earlier; mu half
            # second, its copy/DMA overlapping tanh/exp.
            # separate PSUM tiles for the two halves: a shared tile would
            # put a false reader-reader dep between tanh and the mu copy
            ls_ps = ppool.tile([BS_LOCAL, OUT_W], fp32)
            mu_ps = ppool.tile([BS_LOCAL, OUT_W], fp32)
            nc.tensor.matmul(
                ls_ps[:],
                m[:],
                wp[:, C_WM2 + OUT_W : C_WM2 + 2 * OUT_W],
                start=True,
                stop=not with_bias,
            )
            if with_bias:
                nc.tensor.matmul(
                    ls_ps[:],
                    ones[:],
                    wp[0:1, C_BM2 : C_BM2 + OUT_W],
                    start=False,
                    stop=True,
                )
            nc.tensor.matmul(
                mu_ps[:],
                m[:],
                wp[:, C_WM2 : C_WM2 + OUT_W],
                start=True,
                stop=True,
            )

            # std = exp(3.5*tanh(ls) - 1.5); DMA from ACT right after exp
            O = pool.tile([BS_LOCAL, 2 * OUT_W], fp32)
            tls = pool.tile([BS_LOCAL, OUT_W], fp32)
            nc.scalar.activation(tls[:], ls_ps[:], AF.Tanh)
            nc.vector.tensor_copy(O[:, 0:OUT_W], mu_ps[:])
            nc.sync.dma_start(out[0], O[:, 0:OUT_W])
            nc.scalar.activation(
                O[:, OUT_W : 2 * OUT_W], tls[:], AF.Exp, bias=cm15[:], scale=3.5
            )
            nc.scalar.dma_start(out[1], O[:, OUT_W : 2 * OUT_W])

    nc.compile()
    return nc


def _get_nc(with_bias):
    key = ("bias" if with_bias else "fast")
    if key not in _NC_CACHE:
        _NC_CACHE[key] = _build_bass(with_bias)
    return _NC_CACHE[key]


def _prep_inputs(inputs):
    import ml_dtypes

    bf16 = ml_dtypes.bfloat16

    obs = np.asarray(inputs["obs"], dtype=np.float32)
    W1 = np.asarray(inputs["W1"], dtype=np.float32)
    b1 = np.asarray(inputs["b1"], dtype=np.float32)
    W2 = np.asarray(inputs["W2"], dtype=np.float32)
    b2 = np.asarray(inputs["b2"], dtype=np.float32)
    Wm1 = np.asarray(inputs["Wm1"], dtype=np.float32)
    bm1 = np.asarray(inputs["bm1"], dtype=np.float32)
    Wm2 = np.asarray(inputs["Wm2"], dtype=np.float32)
    bm2 = np.asarray(inputs["bm2"], dtype=np.float32)

    # GCN symmetric norm over the complete-graph-with-self-loops: 1/32 per
    # edge; folded into W1. Layer 2's mean over identical node features is
    # the identity, so W2 is used as-is. w1til8 tiles W1 rows 8x so mm1
    # (K=128) also performs the final 8-way node sum of the pooling tree.
    w1b = np.zeros((FD, H), np.float32)
    w1b[2:FD] = W1 * np.float32(1.0 / 32.0)     # drops robot_loc cols 0:2
    w1til8 = np.tile(w1b, (8, 1))               # [128, 128]

    Wm2r = np.concatenate([np.tile(Wm2[:, 0:2], NN), np.tile(Wm2[:, 2:4], NN)], axis=1)
    bm2pad = np.zeros((H, OUT_W), np.float32)
    bm2pad[0] = np.tile(bm2[2:4], NN)           # log_std-plane bias row

    wpk = np.ascontiguousarray(
        np.concatenate([W2, Wm1, Wm2r, bm2pad], axis=1).astype(bf16)
    )
    bcols = np.ascontiguousarray(
        np.stack([b1, b2, bm1, np.zeros(H, np.float32)], axis=1)
    )

    with_bias = bool(
        np.any(b1) or np.any(b2) or np.any(bm1) or np.any(bm2[2:4])
    )
    obs16 = np.ascontiguousarray(obs.astype(bf16))
    shared = {"wpk": wpk, "w1t": np.ascontiguousarray(w1til8.astype(bf16))}
    if with_bias:
        shared["bcols"] = bcols
    in_maps = []
    for c in range(NCORES):
        mm = dict(shared)
        mm["obs"] = obs16[c * BS_LOCAL : (c + 1) * BS_LOCAL]
        in_maps.append(mm)
    return in_maps, with_bias


def kernel(**inputs):
    from concourse.bass_utils import run_bass_kernel_spmd

    assert inputs["obs"].shape == (BS, OBS_W), inputs["obs"].shape
    in_maps, with_bias = _prep_inputs(inputs)
    nc = _get_nc(with_bias)
    try:
        res = run_bass_kernel_spmd(nc, in_maps, list(range(NCORES))).results
    except Exception:
        # one retry: absorbs rare transient NRT device hiccups
        res = run_bass_kernel_spmd(nc, in_maps, list(range(NCORES))).results
    out = np.empty((2, BS, OUT_W), np.float32)
    for c in range(NCORES):
        out[:, c * BS_LOCAL : (c + 1) * BS_LOCAL, :] = res[c]["out"]
    # mu-plane bm2 is outside every nonlinearity -> exact host add
    bm2 = np.asarray(inputs["bm2"], dtype=np.float32)
    if bm2[0] != 0.0 or bm2[1] != 0.0:
        out[0] += np.tile(bm2[0:2], NN)[None, :]
    return out


# revision 22
# speedup vs baseline: 1.0407x; 1.0322x over previous
"""Trainium2 Bass kernel for nn_GCNNDiagGaussianActor.

Structural insight: the reference GNN runs GCNConv layers over a COMPLETE
graph of 32 nodes per sample with self-loops. Every node has degree exactly
32 and the symmetric GCN norm is 1/32 for every edge, so the gather +
segment_sum collapses to a per-graph mean broadcast to every node. Per graph:

    pooled = sum_n obs[g, n, 0:16]                   (cols 0:2 zeroed in W1)
    h1  = relu(pooled @ (W1/32) + b1)
    h2  = relu(h1 @ W2 + b2)
    m   = relu(h2 @ Wm1 + bm1)
    o   = m @ Wm2r + bm2r                            -> [128] per graph
    mu  = o[0:64];  std = exp(3.5 * tanh(o[64:128]) - 1.5)

Sharding: data-parallel over batch, 128 graphs per core = 128 SBUF
partitions; small weights replicated. The x32 node replication is folded
into the last GEMM by tiling Wm2's columns host-side.

Perf structure (v3 fp32 baseline ~22.3us -> ~17.1us measured same-state;
the ~13us floor is fixed NEFF overhead: ~6.5us wrapper semaphore-clear
epilogue + ~1.1us start barrier + ~2.2us minimum latency per DMA chain,
measured empirically with a trivial copy kernel):
- whole datapath in bf16 (harness rel-err gate is 2e-2; this kernel lands
  ~1e-3): obs DMA halves to 128KB/core and every matmul runs 1 cycle/row
  instead of fp32's 4 (no LOW_HIGH double pass).
- obs is split 72/56 across the SP and ACT HWDGE queue sets: the two
  ~630ns issue costs AND the transfers run in parallel (skew compensates
  ACT's ~134ns-slower DGE kick). mm1's weights ride second on SP (small,
  lands just before mm1); the fat W2|Wm1|Wm2r pack rides Pool's SWDGE,
  whose ~1us generation overlaps the window start and whose consumers
  (mm2+) run ~1.5us after mm1.
- node pooling: two contiguous bf16 adds (512->256->128) on DVE, then ONE
  PE transpose of the [128,128] partial-sum block into PSUM, a DVE bf16
  copy back to SBUF, and the remaining 8-way node sum folded into the
  first matmul by host-tiling W1 rows to K=128 (replaces the strided
  reduce + 4 stream transposes of earlier versions, ~1us faster).
- the transpose identity is built on-device (Pool iota, DVE is_equal)
  so it never competes with input DMA bandwidth.
- zero-bias fast path (this workload's biases are all zeros): relus are
  bare tensor_scalar_max on DVE. General path keeps b1/b2/bm1 as fp32
  bias columns fused into the relus and bm2 as a rank-1 PE accumulate.
- final GEMM split into log_std and mu halves on separate PSUM tiles
  (a shared tile creates a false reader-reader dep): tanh fires right
  after the ls half; the mu copy + SP DMA overlap tanh/exp; the std DMA
  issues from ACT the moment exp retires.
- PE p-state pre-warm: dummy matmuls on a junk tile keep the tensor
  engine busy through the obs DMA wait so real matmuls run at high clock.
- dummy tanh right after the DMA issues hoists the scalar engine's
  ACT_TABLE_LOAD (~1.3us) off the critical path.
- custom TileContext exit: drop the on-device semaphore RANGE_CLEAR and
  second exit barrier (the NEFF wrapper epilogue clears every semaphore
  after the kernel anyway), ~0.4us off the tail.
"""

import numpy as np

NCORES = 8
BS = 1024
BS_LOCAL = BS // NCORES   # 128 graphs per core
NN = 32                   # nodes per graph
FD = 16                   # per-node obs width
OBS_W = NN * FD           # 512
H = 128                   # hidden width
OUT_W = 2 * NN            # 64 = ACT_DIM * NN
# wpk cols: W2 | Wm1 | Wm2r | bm2ls row (w1til8 ships separately on SP;
# the transpose identity is built on-device)
C_W2 = 0
C_WM1 = H
C_WM2 = 2 * H
C_BM2 = 3 * H            # row 0 only, 64 wide
WPK = 3 * H + OUT_W
N_WARM_MM = 4             # PE p-state warm-up matmuls

_NC_CACHE = {}


def _build_bass(with_bias):
    import concourse.bacc as bacc
    import concourse.mybir as mybir
    from concourse import tile
    from concourse.vector_clock import ScopedClock

    class FastExitTileContext(tile.TileContext):
        # The stock exit emits drain + barrier + gpsimd sem RANGE_CLEAR +
        # second barrier. The NEFF wrapper epilogue clears every semaphore
        # after the kernel regardless, so the on-device clear and the
        # second barrier are pure tail latency here; keep only the
        # python-side allocator bookkeeping. post_barrier_emit lets the
        # kernel issue its output DMAs AFTER the exit barrier: the barrier
        # already proves the data is in SBUF, so the DMAs need no
        # semaphores and their ~2.2us latency (DGE kick + transfer +
        # completion-sem propagation) overlaps the wrapper's ~6.5us
        # semaphore-clear teardown instead of preceding it.
        post_barrier_emit = None

        def _drain_and_barrier(self, tick_clock, wait_clock):
            drain_inst = self.nc.sync.drain()
            wait_clock.add_sem_waits(
                drain_inst.ins, ScopedClock({None: tick_clock.global_clock})
            )
            self.nc.all_engine_barrier()
            popped = self.nc._tile_sem_poison_stack.pop()
            assert popped is self._sem_poison
            sems = list(self.sems.allocated().values())
            if sems:
                sem_nums = [
                    s.num if hasattr(s, "num") else s for s in sems
                ]
                self.nc._state.prepend_free_semaphores(sem_nums)
                for poison_set in self.nc._tile_sem_poison_stack:
                    poison_set.update(sem_nums)
            if self.post_barrier_emit is not None:
                self.post_barrier_emit()

    fp32 = mybir.dt.float32
    bf16 = mybir.dt.bfloat16
    AF = mybir.ActivationFunctionType
    ALU = mybir.AluOpType

    nc = bacc.Bacc(None, target_bir_lowering=False)
    obs = nc.declare_dram_parameter("obs", [BS_LOCAL, OBS_W], bf16, isOutput=False)
    wpk = nc.declare_dram_parameter("wpk", [H, WPK], bf16, isOutput=False)
    w1t = nc.declare_dram_parameter("w1t", [H, H], bf16, isOutput=False)
    if with_bias:
        bcols = nc.declare_dram_parameter("bcols", [H, 4], fp32, isOutput=False)
    out = nc.declare_dram_parameter("out", [2, BS_LOCAL, OUT_W], fp32, isOutput=True)
    # completion sem for the post-barrier output DMAs: never waited on by
    # this kernel (the wrapper teardown outlasts the transfers), but walrus
    # codegen requires dynamic DMAs to carry a semaphore update
    out_sem = nc.alloc_semaphore("out_dma_sem")

    with FastExitTileContext(nc) as tc:
        with (
            tc.tile_pool(name="sb", bufs=1) as pool,
            tc.tile_pool(name="ps", bufs=1, space="PSUM") as ppool,
        ):
            obs_t = pool.tile([BS_LOCAL, OBS_W], bf16)
            wp = pool.tile([H, WPK], bf16)
            w1_t = pool.tile([H, H], bf16)
            # obs split across SP and ACT HWDGE queue sets (parallel issue
            # AND parallel transfer); mm1's weights ride second on SP (small,
            # arrives just before mm1 needs them); the rest of the weights
            # ride Pool's SWDGE, whose ~1us generation overlaps the window
            # start and whose consumers (mm2+) run later
            HB = 72  # skewed split: ACT's DGE kick is ~134ns slower than SP's
            nc.sync.dma_start(obs_t[0:HB, :], obs[0:HB, :])
            nc.scalar.dma_start(obs_t[HB:BS_LOCAL, :], obs[HB:BS_LOCAL, :])
            nc.sync.dma_start(w1_t[:], w1t[:])
            nc.gpsimd.dma_start(wp[:], wpk[:])
            if with_bias:
                bc = pool.tile([H, 4], fp32)
                nc.sync.dma_start(bc[:], bcols[:])

            # transpose identity built on-device: Pool iota (col - row),
            # DVE is_equal -> 1.0 on the diagonal; both engines idle here
            iot = pool.tile([H, H], mybir.dt.int32)
            nc.gpsimd.iota(iot[:], pattern=[[1, H]], channel_multiplier=-1)
            ident = pool.tile([H, H], bf16)
            nc.vector.tensor_scalar(ident[:], iot[:], 0, None, ALU.is_equal)

            # junk tile for PE warm-up + small constants (DVE, off-path)
            junk = pool.tile([BS_LOCAL, OBS_W], bf16)
            nc.vector.memset(junk[:], 0.0)
            if with_bias:
                ones = pool.tile([1, H], bf16)
                nc.vector.memset(ones[:], 1.0)
            cm15 = pool.tile([BS_LOCAL, 1], fp32)
            nc.vector.memset(cm15[:], -1.5)
            warm = pool.tile([1, 1], fp32)
            nc.vector.memset(warm[:], 0.0)
            # dummy transcendental: hoists ACT_TABLE_LOAD into the DMA wait
            nc.scalar.activation(warm[:], warm[:], AF.Tanh)

            # PE p-state warm-up during the obs DMA wait; the last short
            # matmul keeps PE busy up to the A2 transpose arrival
            jp = ppool.tile([1, OBS_W], fp32)
            for _ in range(N_WARM_MM):
                nc.tensor.matmul(jp[:], junk[:, 0:1], junk[:], start=True, stop=True)
            nc.tensor.matmul(jp[:, 0:128], junk[:, 0:1], junk[:, 0:128], start=True, stop=True)

            # Node pooling, stage 1: 512 -> 256 -> 128 contiguous bf16 adds.
            # A2[g, n1*16+d] = sum over nodes n = n1 (mod 8) of obs dim d.
            A1 = pool.tile([BS_LOCAL, 256], bf16)
            nc.vector.tensor_add(A1[:], obs_t[:, 0:256], obs_t[:, 256:512])
            A2 = pool.tile([BS_LOCAL, 128], bf16)
            nc.vector.tensor_add(A2[:], A1[:, 0:128], A1[:, 128:256])

            # Stage 2: one PE transpose (vs 4 DVE stream transposes), then
            # the 8-way node sum rides inside mm1 via host-tiled W1 (K=128).
            tp = ppool.tile([H, BS_LOCAL], bf16)
            nc.tensor.transpose(tp[:], A2[:], ident[:])
            B = pool.tile([H, BS_LOCAL], bf16)
            nc.vector.tensor_copy(B[:], tp[:])

            # MLP chain: [ch, graphs] tiles, weights as lhsT, relu+bias
            # fused on DVE (out = max(psum + b, 0)).
            def relu(dst, src, col):
                if with_bias:
                    nc.vector.tensor_scalar(
                        dst, src, bc[:, col : col + 1], 0.0, ALU.add, ALU.max
                    )
                else:
                    nc.vector.tensor_scalar_max(dst, src, 0.0)

            h1_ps = ppool.tile([H, BS_LOCAL], fp32)
            nc.tensor.matmul(h1_ps[:], w1_t[:], B[:], start=True, stop=True)
            h1 = pool.tile([H, BS_LOCAL], bf16)
            relu(h1[:], h1_ps[:], 0)

            h2_ps = ppool.tile([H, BS_LOCAL], fp32)
            nc.tensor.matmul(h2_ps[:], wp[:, C_W2 : C_W2 + H], h1[:], start=True, stop=True)
            h2 = pool.tile([H, BS_LOCAL], bf16)
            relu(h2[:], h2_ps[:], 1)

            m_ps = ppool.tile([H, BS_LOCAL], fp32)
            nc.tensor.matmul(m_ps[:], wp[:, C_WM1 : C_WM1 + H], h2[:], start=True, stop=True)
            m = pool.tile([H, BS_LOCAL], bf16)
            relu(m[:], m_ps[:], 2)

            # Final layer, graphs on PSUM partitions. log_std half first
            # (+ rank-1 bm2 accumulate) so tanh starts earlier; mu half
            # second, its copy/DMA overlapping tanh/exp.
            # separate PSUM tiles for the two halves: a shared tile would
            # put a false reader-reader dep between tanh and the mu copy
            ls_ps = ppool.tile([BS_LOCAL, OUT_W], fp32)
            mu_ps = ppool.tile([BS_LOCAL, OUT_W], fp32)
            nc.tensor.matmul(
                ls_ps[:],
                m[:],
                wp[:, C_WM2 + OUT_W : C_WM2 + 2 * OUT_W],
                start=True,
                stop=not with_bias,
            )
            if with_bias:
                nc.tensor.matmul(
                    ls_ps[:],
                    ones[:],
                    wp[0:1, C_BM2 : C_BM2 + OUT_W],
                    start=False,
                    stop=True,
                )
            nc.tensor.matmul(
                mu_ps[:],
                m[:],
                wp[:, C_WM2 : C_WM2 + OUT_W],
                start=True,
                stop=True,
            )

            # std = exp(3.5*tanh(ls) - 1.5). O is a raw SBUF tensor (not a
            # pool tile) so the post-barrier output DMAs get concrete APs.
            O = nc.alloc_sbuf_tensor("O_out", [BS_LOCAL, 2 * OUT_W], fp32)
            tls = pool.tile([BS_LOCAL, OUT_W], fp32)
            nc.scalar.activation(tls[:], ls_ps[:], AF.Tanh)
            nc.vector.tensor_copy(O[:, 0:OUT_W], mu_ps[:])
            nc.scalar.activation(
                O[:, OUT_W : 2 * OUT_W], tls[:], AF.Exp, bias=cm15[:], scale=3.5
            )

            def _emit_out_dmas():
                nc.sync.dma_start(out[0], O[:, 0:OUT_W]).then_inc(out_sem, 16)
                nc.scalar.dma_start(out[1], O[:, OUT_W : 2 * OUT_W]).then_inc(
                    out_sem, 16
                )

            tc.post_barrier_emit = _emit_out_dmas

    nc.compile()
    return nc


def _get_nc(with_bias):
    key = ("bias" if with_bias else "fast")
    if key not in _NC_CACHE:
        _NC_CACHE[key] = _build_bass(with_bias)
    return _NC_CACHE[key]


def _prep_inputs(inputs):
    import ml_dtypes

    bf16 = ml_dtypes.bfloat16

    obs = np.asarray(inputs["obs"], dtype=np.float32)
    W1 = np.asarray(inputs["W1"], dtype=np.float32)
    b1 = np.asarray(inputs["b1"], dtype=np.float32)
    W2 = np.asarray(inputs["W2"], dtype=np.float32)
    b2 = np.asarray(inputs["b2"], dtype=np.float32)
    Wm1 = np.asarray(inputs["Wm1"], dtype=np.float32)
    bm1 = np.asarray(inputs["bm1"], dtype=np.float32)
    Wm2 = np.asarray(inputs["Wm2"], dtype=np.float32)
    bm2 = np.asarray(inputs["bm2"], dtype=np.float32)

    # GCN symmetric norm over the complete-graph-with-self-loops: 1/32 per
    # edge; folded into W1. Layer 2's mean over identical node features is
    # the identity, so W2 is used as-is. w1til8 tiles W1 rows 8x so mm1
    # (K=128) also performs the final 8-way node sum of the pooling tree.
    w1b = np.zeros((FD, H), np.float32)
    w1b[2:FD] = W1 * np.float32(1.0 / 32.0)     # drops robot_loc cols 0:2
    w1til8 = np.tile(w1b, (8, 1))               # [128, 128]

    Wm2r = np.concatenate([np.tile(Wm2[:, 0:2], NN), np.tile(Wm2[:, 2:4], NN)], axis=1)
    bm2pad = np.zeros((H, OUT_W), np.float32)
    bm2pad[0] = np.tile(bm2[2:4], NN)           # log_std-plane bias row

    wpk = np.ascontiguousarray(
        np.concatenate([W2, Wm1, Wm2r, bm2pad], axis=1).astype(bf16)
    )
    bcols = np.ascontiguousarray(
        np.stack([b1, b2, bm1, np.zeros(H, np.float32)], axis=1)
    )

    with_bias = bool(
        np.any(b1) or np.any(b2) or np.any(bm1) or np.any(bm2[2:4])
    )
    obs16 = np.ascontiguousarray(obs.astype(bf16))
    shared = {"wpk": wpk, "w1t": np.ascontiguousarray(w1til8.astype(bf16))}
    if with_bias:
        shared["bcols"] = bcols
    in_maps = []
    for c in range(NCORES):
        mm = dict(shared)
        mm["obs"] = obs16[c * BS_LOCAL : (c + 1) * BS_LOCAL]
        in_maps.append(mm)
    return in_maps, with_bias


def kernel(**inputs):
    from concourse.bass_utils import run_bass_kernel_spmd

    assert inputs["obs"].shape == (BS, OBS_W), inputs["obs"].shape
    in_maps, with_bias = _prep_inputs(inputs)
    nc = _get_nc(with_bias)
    try:
        res = run_bass_kernel_spmd(nc, in_maps, list(range(NCORES))).results
    except Exception:
        # one retry: absorbs rare transient NRT device hiccups
        res = run_bass_kernel_spmd(nc, in_maps, list(range(NCORES))).results
    out = np.empty((2, BS, OUT_W), np.float32)
    for c in range(NCORES):
        out[:, c * BS_LOCAL : (c + 1) * BS_LOCAL, :] = res[c]["out"]
    # mu-plane bm2 is outside every nonlinearity -> exact host add
    bm2 = np.asarray(inputs["bm2"], dtype=np.float32)
    if bm2[0] != 0.0 or bm2[1] != 0.0:
        out[0] += np.tile(bm2[0:2], NN)[None, :]
    return out
